# revision 9
# baseline (speedup 1.0000x reference)
"""HGNN layer on 8 trn2 NeuronCores.

Math identity exploited: for each relation r,
    segment_sum(hr[cols]*vals, rows) = segment_sum(h_all[cols]*vals, rows) @ W_r.T
so we aggregate RAW features per destination (sparse part) and apply the
128x128 relation matmuls per destination tile afterwards (dense part).

Sharding: destination nodes split across 8 cores (12544 rows each, padded
N=100352).  Edges bucketed by (core, relation, dest-tile-of-128, source
range-of-32768).  Each core gathers source rows with dma_gather (int16
indices => 4 source ranges), builds a val-weighted one-hot per 128-edge
chunk on DVE, and reduces with PE matmuls into PSUM:

    aggT[f,d] += G_chunk[e,f].T @ Sv[e,d]        (per chunk)
    msgT[o,d]  = sum_r W_r @ aggT_r               (PE, W_r.T stationary)
    x          = msgT + hT_local_tile             (DVE)
    outT[j,d]  = relu(W_sel @ x + b_sel)          (PE + ACT bias)

Everything stays transposed (features on partitions) so the ReLU bias is a
per-partition scalar; the host transposes back at the end.
"""

import os

import numpy as np

import bass_rust
import concourse.bass as bass
import concourse.mybir as mybir
import concourse.tile as tile
from concourse import library_config
from concourse.bass_utils import run_bass_kernel_spmd

N_ITEM = 80000
N_USER = 20000
N = N_ITEM + N_USER
D = 128
R = 3
NCORES = 8
SHARD = 12544          # 98 tiles of 128 rows per core
T = SHARD // 128       # 98
NPAD = SHARD * NCORES  # 100352
GRP = 7                # tiles per gather group
NGRP = T // GRP        # 14
RANGE = 32768          # int16 index window
NRANGES = 4
ITEM_TILES = N_ITEM // 128  # 625 (global tiles 0..624 are item rows)


def _split_excess_waits(nc, max_waits=1):
    """walrus here rejects >1 sem-wait per instruction: hoist extras onto
    same-engine NoOps placed immediately before."""
    for f in nc.m.functions:
        for bb in f.blocks:
            new_insts = []
            for inst in bb.instructions:
                si = inst.sync_info
                waits = list(si.on_wait) if si else []
                if len(waits) > max_waits:
                    excess, keep = waits[:-max_waits], waits[-max_waits:]
                    for ci in range(0, len(excess), max_waits):
                        nop = mybir.InstNoOp(
                            name=f"{inst.name}-ws{ci}", ins=[], outs=[])
                        nop.engine = inst.engine
                        nop.sync_info = bass_rust.SyncInfo(
                            on_wait=list(excess[ci:ci + max_waits]), on_update=[])
                        new_insts.append(nop)
                    inst.sync_info = bass_rust.SyncInfo(
                        on_wait=list(keep), on_update=list(si.on_update))
                new_insts.append(inst)
            bb.instructions[:] = new_insts


def _prep_edges(rows, cols, vals):
    """Bucket all edges by (relation, global dest tile, source range); emit
    fixed-capacity slot arrays (idx/dest/val) with data-adaptive chunk counts.
    Returns C (chunks per range), and per-relation slot arrays [784, CT*128].
    """
    NT = NPAD // 128  # 784 global tiles
    counts = np.zeros((R, NT, NRANGES), dtype=np.int64)
    keys, orders = [], []
    for r in range(R):
        t_g = rows[r] // 128
        q = cols[r] >> 15
        key = t_g * NRANGES + q
        order = np.argsort(key, kind="stable")
        keys.append(key)
        orders.append(order)
        cnt = np.bincount(key, minlength=NT * NRANGES)
        counts[r] = cnt.reshape(NT, NRANGES)
    C = np.maximum(1, -(-counts.max(axis=(0, 1)) // 128))  # chunks per range
    CT = int(C.sum())
    off = np.concatenate(([0], np.cumsum(C)))  # chunk offsets per range

    idx_a = np.zeros((R, NT, CT * 128), dtype=np.int16)
    dst_a = np.zeros((R, NT, CT * 128), dtype=np.float32)
    val_a = np.zeros((R, NT, CT * 128), dtype=np.float32)
    for r in range(R):
        order = orders[r]
        key_s = keys[r][order]
        cnt = np.bincount(keys[r], minlength=NT * NRANGES)
        gstart = np.concatenate(([0], np.cumsum(cnt)))
        rank = np.arange(len(order)) - gstart[key_s]
        t_s = key_s // NRANGES
        q_s = key_s % NRANGES
        slot = (off[q_s] * 128 + rank).astype(np.int64)
        idx_a[r, t_s, slot] = (cols[r][order] & (RANGE - 1)).astype(np.int16)
        dst_a[r, t_s, slot] = (rows[r][order] % 128).astype(np.float32)
        val_a[r, t_s, slot] = vals[r][order]
    return C, off, CT, idx_a, dst_a, val_a


LAST_EXEC_NS = None


def kernel(h_item, h_user, W_rel, W_item, b_item, W_user, b_user,
           vals, rows, cols):
    h_item = np.asarray(h_item, np.float32)
    h_user = np.asarray(h_user, np.float32)
    W_rel = np.asarray(W_rel, np.float32)
    W_item = np.asarray(W_item, np.float32)
    b_item = np.asarray(b_item, np.float32)
    W_user = np.asarray(W_user, np.float32)
    b_user = np.asarray(b_user, np.float32)
    vals = np.asarray(vals, np.float32)
    rows = np.asarray(rows)
    cols = np.asarray(cols)

    h_pad = np.zeros((NPAD, D), np.float32)
    h_pad[:N_ITEM] = h_item
    h_pad[N_ITEM:N] = h_user

    C, off, CT, idx_a, dst_a, val_a = _prep_edges(rows, cols, vals)

    # ---- per-core host arrays -------------------------------------------
    idx_core = idx_a.reshape(R, NCORES, T, CT * 128)
    dst_core = dst_a.reshape(R, NCORES, T, CT * 128)
    val_core = val_a.reshape(R, NCORES, T, CT * 128)

    NU = NGRP * R                      # 42 units per core
    WIDX = GRP * CT * 8                # idx cols per unit (int16)
    WMETA = GRP * 2 * CT               # dest+val cols per unit

    in_maps = []
    for c in range(NCORES):
        idx_u = np.zeros((NU, 128, WIDX), np.int16)
        meta_u = np.zeros((NU, 128, WMETA), np.float32)
        for g in range(NGRP):
            for r in range(R):
                u = g * R + r
                col0 = 0
                for q in range(NRANGES):
                    cq = int(C[q])
                    sl = slice(off[q] * 128, (off[q] + cq) * 128)
                    e = idx_core[r, c, g * GRP:(g + 1) * GRP, sl].reshape(-1)
                    wr = e.reshape(-1, 16).T  # [16, E/16]
                    for gg in range(8):
                        idx_u[u, gg * 16:(gg + 1) * 16,
                              col0:col0 + e.size // 16] = wr
                    col0 += e.size // 16
                for i in range(GRP):
                    t = g * GRP + i
                    meta_u[u, :, i * 2 * CT:i * 2 * CT + CT] = \
                        dst_core[r, c, t].reshape(CT, 128).T
                    meta_u[u, :, i * 2 * CT + CT:(i + 1) * 2 * CT] = \
                        val_core[r, c, t].reshape(CT, 128).T
        hT = np.ascontiguousarray(h_pad[c * SHARD:(c + 1) * SHARD].T)
        gt0 = c * T  # first global tile of this core
        m = (np.arange(gt0, gt0 + T) >= ITEM_TILES).astype(np.float32)
        in_maps.append({
            **{f"h_tab{q}": np.ascontiguousarray(
                h_pad[q * RANGE:min((q + 1) * RANGE, NPAD)])
               for q in range(NRANGES)},
            "hT": hT,
            "idx": idx_u,
            "meta": meta_u,
            "mask": np.broadcast_to(m, (128, T)).copy(),
            "WrT": np.ascontiguousarray(np.transpose(W_rel, (0, 2, 1))),
            "WioT": np.ascontiguousarray(W_item.T),
            "dWT": np.ascontiguousarray((W_user - W_item).T),
            "bI": b_item.reshape(128, 1).copy(),
            "dB": (b_user - b_item).reshape(128, 1).copy(),
            "iota": np.broadcast_to(
                np.arange(128, dtype=np.float32), (128, 128)).copy(),
        })

    # ---- device program --------------------------------------------------
    f32, i16 = mybir.dt.float32, mybir.dt.int16
    nc = bass.Bass("TRN2", target_bir_lowering=False, debug=False,
                   num_devices=NCORES)
    d_tabs = [nc.dram_tensor(
        f"h_tab{q}", [min((q + 1) * RANGE, NPAD) - q * RANGE, D], f32,
        kind="ExternalInput").ap() for q in range(NRANGES)]
    d_hT = nc.dram_tensor("hT", [128, SHARD], f32, kind="ExternalInput").ap()
    d_idx = nc.dram_tensor("idx", [NU, 128, WIDX], i16,
                           kind="ExternalInput").ap()
    d_meta = nc.dram_tensor("meta", [NU, 128, WMETA], f32,
                            kind="ExternalInput").ap()
    d_mask = nc.dram_tensor("mask", [128, T], f32, kind="ExternalInput").ap()
    d_WrT = nc.dram_tensor("WrT", [R, 128, 128], f32,
                           kind="ExternalInput").ap()
    d_WioT = nc.dram_tensor("WioT", [128, 128], f32,
                            kind="ExternalInput").ap()
    d_dWT = nc.dram_tensor("dWT", [128, 128], f32, kind="ExternalInput").ap()
    d_bI = nc.dram_tensor("bI", [128, 1], f32, kind="ExternalInput").ap()
    d_dB = nc.dram_tensor("dB", [128, 1], f32, kind="ExternalInput").ap()
    d_iota = nc.dram_tensor("iota", [128, 128], f32,
                            kind="ExternalInput").ap()
    d_out = nc.dram_tensor("outT", [128, SHARD], f32,
                           kind="ExternalOutput").ap()

    AL = mybir.AluOpType
    with tile.TileContext(nc) as tc:
        with tc.tile_pool(name="const", bufs=1) as cpool, \
             tc.tile_pool(name="gpool", bufs=2) as gpool, \
             tc.tile_pool(name="spool", bufs=16) as spool, \
             tc.tile_pool(name="apool", bufs=2) as apool, \
             tc.tile_pool(name="opool", bufs=4) as opool, \
             tc.tile_pool(name="ppool", bufs=2, space="PSUM") as ppool, \
             tc.tile_pool(name="p2", bufs=2, space="PSUM") as p2, \
             tc.tile_pool(name="p3", bufs=2, space="PSUM") as p3:

            nc.gpsimd.load_library(library_config.mlp)
            nidx_regs = {}
            for q in range(NRANGES):
                nv = GRP * int(C[q]) * 128
                if nv not in nidx_regs:
                    nidx_regs[nv] = nc.gpsimd.to_reg(nv)

            iota_t = cpool.tile([128, 128], f32, tag="iota")
            nc.sync.dma_start(out=iota_t[:], in_=d_iota[:])
            mask_t = cpool.tile([128, T], f32, tag="mask")
            nc.sync.dma_start(out=mask_t[:], in_=d_mask[:])
            wr_t = []
            for r in range(R):
                w = cpool.tile([128, 128], f32, tag=f"WrT{r}")
                nc.sync.dma_start(out=w[:], in_=d_WrT[r, :, :])
                wr_t.append(w)
            wio_t = cpool.tile([128, 128], f32, tag="WioT")
            nc.sync.dma_start(out=wio_t[:], in_=d_WioT[:])
            dw_t = cpool.tile([128, 128], f32, tag="dWT")
            nc.sync.dma_start(out=dw_t[:], in_=d_dWT[:])
            bI_t = cpool.tile([128, 1], f32, tag="bI")
            nc.sync.dma_start(out=bI_t[:], in_=d_bI[:])
            dB_t = cpool.tile([128, 1], f32, tag="dB")
            nc.sync.dma_start(out=dB_t[:], in_=d_dB[:])

            for g in range(NGRP):
                agg = [[apool.tile([128, 128], f32, name=f"agg_{g}_{r}_{i}",
                                   tag=f"agg{r}_{i}")
                        for i in range(GRP)] for r in range(R)]
                for r in range(R):
                    u = g * R + r
                    idx_t = gpool.tile([128, WIDX], i16, tag="idx")
                    nc.sync.dma_start(out=idx_t[:], in_=d_idx[u, :, :])
                    meta_t = gpool.tile([128, WMETA], f32, tag="meta")
                    nc.sync.dma_start(out=meta_t[:], in_=d_meta[u, :, :])
                    G = gpool.tile([128, GRP * CT, 128], f32, tag="G")
                    col0 = 0
                    for q in range(NRANGES):
                        cq = int(C[q])
                        nidx = GRP * cq * 128
                        nc.gpsimd.dma_gather(
                            out_ap=G[:, off[q] * GRP:(off[q] + cq) * GRP, :],
                            in_ap=d_tabs[q][:, :],
                            idxs_ap=idx_t[:, col0:col0 + nidx // 16],
                            num_idxs=nidx,
                            num_idxs_reg=nidx_regs[nidx],
                            elem_size=D,
                            single_packet=False,
                        )
                        col0 += nidx // 16
                    for i in range(GRP):
                        pA = ppool.tile([128, 128], f32, tag="pA")
                        k = 0
                        for q in range(NRANGES):
                            for ci in range(int(C[q])):
                                ch = off[q] * GRP + i * int(C[q]) + ci
                                j = off[q] + ci
                                sv = spool.tile([128, 128], f32, tag="sv")
                                dcol = meta_t[:, i * 2 * CT + j:
                                              i * 2 * CT + j + 1]
                                vcol = meta_t[:, i * 2 * CT + CT + j:
                                              i * 2 * CT + CT + j + 1]
                                nc.vector.tensor_scalar(
                                    out=sv[:], in0=iota_t[:],
                                    scalar1=dcol, scalar2=vcol,
                                    op0=AL.is_equal, op1=AL.mult)
                                nc.tensor.matmul(
                                    out=pA[:], lhsT=G[:, ch, :], rhs=sv[:],
                                    start=(k == 0), stop=(k == CT - 1))
                                k += 1
                        nc.vector.tensor_copy(
                            out=agg[r][i][:], in_=pA[:])
                for i in range(GRP):
                    t = g * GRP + i
                    pM = p2.tile([128, 128], f32, tag="pM")
                    for r in range(R):
                        nc.tensor.matmul(
                            out=pM[:], lhsT=wr_t[r][:], rhs=agg[r][i][:],
                            start=(r == 0), stop=(r == R - 1))
                    ht = opool.tile([128, 128], f32, tag="ht")
                    nc.sync.dma_start(out=ht[:],
                                      in_=d_hT[:, t * 128:(t + 1) * 128])
                    x = opool.tile([128, 128], f32, tag="x")
                    nc.vector.tensor_tensor(
                        out=x[:], in0=pM[:], in1=ht[:], op=AL.add)
                    wе = opool.tile([128, 128], f32, tag="we")
                    nc.vector.tensor_scalar(
                        out=wе[:], in0=dw_t[:],
                        scalar1=mask_t[:, t:t + 1], scalar2=None, op0=AL.mult)
                    nc.vector.tensor_tensor(
                        out=wе[:], in0=wе[:], in1=wio_t[:], op=AL.add)
                    be = opool.tile([128, 1], f32, tag="be")
                    nc.vector.tensor_scalar(
                        out=be[:], in0=dB_t[:],
                        scalar1=mask_t[:, t:t + 1], scalar2=None, op0=AL.mult)
                    nc.vector.tensor_tensor(
                        out=be[:], in0=be[:], in1=bI_t[:], op=AL.add)
                    pO = p3.tile([128, 128], f32, tag="pO")
                    nc.tensor.matmul(out=pO[:], lhsT=wе[:], rhs=x[:],
                                     start=True, stop=True)
                    o = opool.tile([128, 128], f32, tag="o")
                    nc.scalar.activation(
                        out=o[:], in_=pO[:],
                        func=mybir.ActivationFunctionType.Relu,
                        bias=be[:, :1], scale=1.0)
                    nc.sync.dma_start(
                        out=d_out[:, t * 128:(t + 1) * 128], in_=o[:])

    _split_excess_waits(nc)
    mybir.codegen_inst_isa_subclasses(nc)

    import time as _time
    _t0 = _time.time()
    res = run_bass_kernel_spmd(nc, in_maps, core_ids=list(range(NCORES)))
    global LAST_EXEC_NS
    LAST_EXEC_NS = res.exec_time_ns
    if LAST_EXEC_NS is None:
        # NTFF hook unavailable in this container: report the full PJRT
        # dispatch wall (includes input upload + execute + output fetch) as
        # an upper bound.
        LAST_EXEC_NS = int((_time.time() - _t0) * 1e9)
    full = np.empty((NPAD, D), np.float32)
    for c in range(NCORES):
        full[c * SHARD:(c + 1) * SHARD] = res.results[c]["outT"].T
    return full[:N_ITEM].copy(), full[N_ITEM:N].copy()
